# revision 42
# baseline (speedup 1.0000x reference)
"""Trainium2 Bass kernel for the grouped 5D-score attention layer.

Problem (all fp32):
  query [I=128, Q=32, 64], key [T=128, K=32, 64], key_padding_mask [T, K] bool,
  W_Q/W_K/W_V [16,16], W_fc [64,64], HEADS=4, HEAD_DIM=16.
  scores[i,t,h,q,k] = (qn[i,h,q] . kn[t,h,k]) / 8, masked over (t,k),
  p = softmax_k(scores); va = p @ vn; out[i,t,q,:] = (va reshaped) @ W_fc.T
  Output: [128, 128, 32, 64].

Sharding: i-axis split across 8 cores (16 i per core). Key-side tensors are
replicated. No collectives.

Per-core device pipeline, streaming over t-pairs (iq = 16*32 = 512 columns):
  1. QK matmuls (fp32r) into a pair tile sc2 [128, 1024]: lhsT = KB_t
     [65, 128] block-diag of kn per head plus a mask-bias row (paired with a
     ones-row in the rhs), rhs = QN_aug [65, 512].
  2. One ACT exp per pair (Exp/Copy are the only ACT functions and share a
     table set -> the activation table loads exactly once; the v1 kernel's
     Exp/Reciprocal alternation reloaded it twice per pair at 1283ns each).
  3. sbc matmuls (ones block-diag) -> S broadcast, 2-t packed [128, 512]
     PSUM; DVE nc.vector.reciprocal reads it straight from PSUM -> rs SBUF
     (TensorTensor divide is not a valid HW op; GPSIMD cannot access PSUM).
  4. va matmuls (vb block-diag) -> va2 2-t packed [128, 512] PSUM;
     DVE multiply va2*rs -> van2 bf16 SBUF.
  5. fc: ONE matmul with the constant blockdiag(W_fc.T) stationary, van2
     moving -> u2T [(par e), (i q)] f32 PSUM (transposed output; fewer
     Ldweights).  ACT Copy stages it to SBUF; one 256KB DMA per pair; host
     permutes.

Software pipeline, slot n:
  PE : QK(n+1) x2, sbc(n) x2, va(n) x2, fc(n-2)
  ACT: exp(n+1), u2cp(n-2)
  DVE: recip(n-1), mult(n-1)
  SP : store(n-2)
Every instruction's inputs are at least one slot old, so no engine stream
ever stalls mid-slot on same-slot work from another engine (the ACT->PE->ACT
round trip was the previous 2us/pair binder).
"""

import sys
from contextlib import ExitStack

import numpy as np

sys.path.insert(0, "/opt/trn_rl_repo")

import concourse.bass as bass  # noqa: E402
import concourse.tile as tile  # noqa: E402
from concourse import mybir  # noqa: E402
from concourse.bass_utils import run_bass_kernel_spmd  # noqa: E402

# Problem constants (hardcoded; kernel.py must be self-contained).
D_MODEL = 64
HEADS = 4
HD = 16  # head dim
I, Q = 128, 32
T, K = 128, 32
N_CORES = 8
I_SH = I // N_CORES  # 16 i-groups per core
IQ = I_SH * Q  # 512 moving columns
SCALE = 1.0 / 8.0  # 1/sqrt(D_MODEL)
MASK_NEG = -30000.0  # exp(-30000) == 0 in fp32

F32 = mybir.dt.float32
F32R = mybir.dt.float32r
BF16 = mybir.dt.bfloat16

TCH = 16  # t-chunk for KB/VB input DMA batching

# "dve": rs = nc.vector.reciprocal(S)      (DVE iterative divide)
# "lnexp": rs = exp(-ln(S)) on ACT         (ln/exp share one table set)
RECIP_MODE = "dve"


def _split_excess_matmul_waits(nc):
    """This walrus build gives engine instructions a single sync-wait slot.
    Hoist extra waits onto an inserted same-engine NoOp (the sequencer
    executes the NoOp's waits before dispatching the real instruction)."""
    n_split = 0
    for blk in nc.main_func.blocks:
        insts = blk.instructions
        i = 0
        while i < len(insts):
            inst = insts[i]
            si = getattr(inst, "sync_info", None)
            if (
                si is not None
                and len(si.on_wait) > 1
                and not isinstance(inst, mybir.InstNoOp)
            ):
                for w in list(si.on_wait[:-1]):
                    nop = mybir.InstNoOp(
                        name=f"I-waitsplit-{n_split}", ins=[], outs=[]
                    )
                    nop.engine = inst.engine
                    nop.sync_info = mybir.SyncInfo(on_wait=[w], on_update=[])
                    nc.register_instruction(nop)
                    insts.insert(i, nop)
                    n_split += 1
                    i += 1
                si.on_wait = si.on_wait[-1:]
            i += 1


def build_kernel_nc(repeat=1):
    nc = bass.Bass()

    qn_aug_d = nc.declare_dram_parameter("qn_aug", [65, IQ], F32R, isOutput=False)
    # partition-major: [65 partitions, T, 128] so a 16-t chunk is one
    # contiguous 8KB run per partition (single descriptor per partition).
    kb_d = nc.declare_dram_parameter("kb", [65, T, 128], F32R, isOutput=False)
    vb_d = nc.declare_dram_parameter("vb", [128, T, 64], BF16, isOutput=False)
    ones_d = nc.declare_dram_parameter("onesbd", [128, 64], BF16, isOutput=False)
    wfc_d = nc.declare_dram_parameter("wfcbd", [128, 128], BF16, isOutput=False)
    # device-natural layout: [t-pair, (i4 q), chunk, parity, e]; host permutes
    out_d = nc.declare_dram_parameter("out", [T // 2, 128, 512], F32, isOutput=True)

    with ExitStack() as ctx:
        tc = ctx.enter_context(tile.TileContext(nc))
        singles = ctx.enter_context(tc.tile_pool(name="singles", bufs=1))
        kvload = ctx.enter_context(tc.tile_pool(name="kvload", bufs=3))
        eps = ctx.enter_context(tc.tile_pool(name="eps", bufs=3))
        rss = ctx.enter_context(tc.tile_pool(name="rss", bufs=2))
        vans = ctx.enter_context(tc.tile_pool(name="vans", bufs=3))
        outs = ctx.enter_context(tc.tile_pool(name="outs", bufs=4))
        ps_sc = ctx.enter_context(tc.tile_pool(name="ps_sc", bufs=2, space="PSUM"))
        # sbc2/va2 share one ring (same shape/dtype, 2 allocs per pair,
        # freed ~1 pair later); u2 lives <1 pair.  2*2 + 3 + 1 = 8 banks.
        ps_sv = ctx.enter_context(tc.tile_pool(name="ps_sv", bufs=3, space="PSUM"))
        ps_u = ctx.enter_context(tc.tile_pool(name="ps_u", bufs=1, space="PSUM"))

        # singles DMAs are interleaved into the first chunk load below so the
        # first QK's inputs (kb chunk + qn) lead the serialized DGE queue
        qn_sb = singles.tile([65, IQ], F32R)
        ones_sb = singles.tile([128, 64], BF16)
        wfc_sb = singles.tile([128, 128], BF16)
        singles_pending = [
            (qn_sb, qn_aug_d),
            (ones_sb, ones_d),
            (wfc_sb, wfc_d),
        ]

        n_pairs = T // 2

        def load_chunk(tc0):
            kb_sb = kvload.tile([65, TCH, 128], F32R, tag="kb")
            nc.sync.dma_start(out=kb_sb, in_=kb_d[:, tc0 : tc0 + TCH, :])
            if singles_pending:
                sb, d = singles_pending.pop(0)
                nc.sync.dma_start(out=sb, in_=d[:, :])
            vb_sb = kvload.tile([128, TCH, 64], BF16, tag="vb")
            nc.sync.dma_start(out=vb_sb, in_=vb_d[:, tc0 : tc0 + TCH, :])
            while singles_pending:
                sb, d = singles_pending.pop(0)
                nc.sync.dma_start(out=sb, in_=d[:, :])
            return kb_sb, vb_sb

        def qk(tp, kb_sb, tc0):
            sc2 = ps_sc.tile([128, 2 * IQ], F32, tag="sc")
            for par in (0, 1):
                t = 2 * tp + par
                nc.tensor.matmul(
                    sc2[:, par * IQ : (par + 1) * IQ],
                    lhsT=kb_sb[:, t - tc0, :],
                    rhs=qn_sb,
                    start=True,
                    stop=True,
                )
            return sc2

        def do_fc(pend):
            # ONE matmul with the constant wfcbd as stationary: out is
            # transposed, u2T [(par, e), (i q)] -- fewer Ldweights than the
            # 4-chunk van2-stationary form; host permute absorbs the layout.
            pvan2, ptp = pend
            u2 = ps_u.tile([128, 4 * 128], F32, tag="u2")
            nc.tensor.matmul(u2, lhsT=wfc_sb, rhs=pvan2, start=True, stop=True)
            # u2 copy on ACT (Copy shares Exp's table set; DVE is saturated
            # by recip+mult)
            ou2 = outs.tile([128, 4 * 128], F32, tag="ou2")
            nc.scalar.activation(ou2, u2, mybir.ActivationFunctionType.Copy)
            nc.sync.dma_start(out=out_d[ptp, :, :], in_=ou2)

        for _rep in range(repeat):
            # Staged software pipeline. In slot n:
            #   PE : QK(n+1), sbc(n), va(n), fc(n-2)
            #   ACT: exp(n+1), Scp(n-1)   -- both inputs a full slot old, so
            #        the ACT stream never stalls on same-slot PE work (the
            #        ACT->PE->ACT cycle was the previous 2us/pair binder)
            #   DVE: u2cp(n-2), div(n-1)
            #   SP : store(n-2)
            chunks = {0: load_chunk(0)}
            sc2 = qk(0, chunks[0][0], 0)
            ep0 = eps.tile([128, 2 * IQ], BF16, tag="ep", name="ep_next")
            nc.scalar.activation(ep0, sc2, mybir.ActivationFunctionType.Exp)
            ep = {0: ep0}
            sbc_t, va_t, ssb, van, fc_pend = {}, {}, {}, {}, {}

            for n in range(n_pairs + 2):
                live = n < n_pairs

                # PE: next pair's QK (+ chunk prefetch)
                if live and n + 1 < n_pairs:
                    ntc0 = (2 * (n + 1)) // TCH * TCH
                    if ntc0 not in chunks:
                        chunks[ntc0] = load_chunk(ntc0)
                        chunks.pop(ntc0 - 2 * TCH, None)
                    nsc2 = qk(n + 1, chunks[ntc0][0], ntc0)

                # DVE: recip(n-1) + mult(n-1) -- inputs a full pair old, so
                # the DVE stream starts each slot without waiting.  The DVE
                # TensorTensor divide is not a valid HW op and GPSIMD cannot
                # access PSUM at all; reciprocal_approx_fast (custom DVE op,
                # ~18 correct bits) reads the S-broadcast straight from PSUM.
                if n - 1 >= 0 and n - 1 < n_pairs:
                    rs = rss.tile([128, IQ], F32, tag="rs")
                    if RECIP_MODE == "dve":
                        nc.vector.reciprocal(out=rs, in_=sbc_t.pop(n - 1))
                    else:
                        lns = rss.tile([128, IQ], F32, tag="lns", name="lns")
                        nc.scalar.activation(
                            lns, sbc_t.pop(n - 1), mybir.ActivationFunctionType.Ln
                        )
                        nc.scalar.activation(
                            rs, lns, mybir.ActivationFunctionType.Exp, scale=-1.0
                        )
                    van2 = vans.tile([128, IQ], BF16, tag="van2")
                    nc.vector.tensor_mul(van2, va_t.pop(n - 1), rs)
                    fc_pend[n - 1] = (van2, n - 1)

                # PE: S broadcast (2-t packed) for pair n
                if live:
                    ep2 = ep[n]
                    sbc2 = ps_sv.tile([128, IQ], F32, tag="sv")
                    for par in (0, 1):
                        nc.tensor.matmul(
                            sbc2[par * 64 : (par + 1) * 64, :],
                            lhsT=ones_sb,
                            rhs=ep2[:, par * IQ : (par + 1) * IQ],
                            start=True,
                            stop=True,
                        )
                    sbc_t[n] = sbc2

                # ACT: exp(n+1) (Copy above shares Exp's table set -> no reload)
                if live and n + 1 < n_pairs:
                    ep_next = eps.tile([128, 2 * IQ], BF16, tag="ep", name="ep_next")
                    nc.scalar.activation(
                        ep_next, nsc2, mybir.ActivationFunctionType.Exp
                    )
                    ep[n + 1] = ep_next



                # PE: va (2-t packed) for pair n
                if live:
                    tc0 = (2 * n) // TCH * TCH
                    _, vb_sb = chunks[tc0]
                    va2 = ps_sv.tile([128, IQ], F32, tag="sv", name="va2")
                    for par in (0, 1):
                        t = 2 * n + par
                        nc.tensor.matmul(
                            va2[par * 64 : (par + 1) * 64, :],
                            lhsT=vb_sb[:, t - tc0, :],
                            rhs=ep2[:, par * IQ : (par + 1) * IQ],
                            start=True,
                            stop=True,
                        )
                    va_t[n] = va2
                    ep.pop(n)

                # PE: fc(n-2); DVE: u2cp(n-2); SP: store(n-2)
                if n - 2 >= 0 and n - 2 < n_pairs:
                    do_fc(fc_pend.pop(n - 2))

    _split_excess_matmul_waits(nc)
    return nc


def _prep_inputs(query, key, key_padding_mask, W_Q, W_K, W_V, W_fc):
    """Host-side marshaling: projections on the small key/query tensors and
    block-diagonal packing into the layouts the device kernel consumes."""
    query = np.asarray(query, dtype=np.float32)
    key = np.asarray(key, dtype=np.float32)
    mask = np.asarray(key_padding_mask)
    W_Q = np.asarray(W_Q, dtype=np.float32)
    W_K = np.asarray(W_K, dtype=np.float32)
    W_V = np.asarray(W_V, dtype=np.float32)
    W_fc = np.asarray(W_fc, dtype=np.float32)
    import ml_dtypes

    # projections (tiny: ~1.5% of model FLOPs; layout-prep for the device)
    q4 = query.reshape(I, Q, HEADS, HD)  # [i,q,h,d]
    k4 = key.reshape(T, K, HEADS, HD)  # [t,k,h,d]
    qn = np.einsum("iqhd,ed->ihqe", q4, W_Q) * SCALE  # [i,h,q,e]
    kn = np.einsum("tkhd,ed->thke", k4, W_K)  # [t,h,k,e]
    vn = np.einsum("tkhd,ed->thke", k4, W_V)  # [t,h,k,e]

    # KB: [T, 65, 128]; rows h*16+e, cols h*32+k block-diag; row 64 = mask bias
    kb = np.zeros((T, 65, 128), dtype=np.float32)
    for h in range(HEADS):
        kb[:, h * HD : (h + 1) * HD, h * K : (h + 1) * K] = kn[:, h].transpose(0, 2, 1)
    kb[:, 64, :] = (
        np.where(mask, np.float32(MASK_NEG), np.float32(0.0))
        .reshape(T, 1, K)
        .repeat(HEADS, axis=1)
        .reshape(T, 128)
    )
    kb_pm = np.ascontiguousarray(kb.transpose(1, 0, 2))  # [65, T, 128]

    # VB: [T, 128, 64]; rows h*32+k, cols h*16+d block-diag of vn (bf16)
    vb = np.zeros((T, 128, 64), dtype=np.float32)
    for h in range(HEADS):
        vb[:, h * K : (h + 1) * K, h * HD : (h + 1) * HD] = vn[:, h]
    vb_pm = np.ascontiguousarray(vb.transpose(1, 0, 2)).astype(
        ml_dtypes.bfloat16
    )  # [128, T, 64]

    # ones block-diag [128, 64]: col h*16+d has ones on rows h*32..h*32+31
    onesbd = np.zeros((128, 64), dtype=np.float32)
    for h in range(HEADS):
        onesbd[h * K : (h + 1) * K, h * HD : (h + 1) * HD] = 1.0
    onesbd = onesbd.astype(ml_dtypes.bfloat16)

    # wfcbd [128, 128] = blockdiag(W_fc.T, W_fc.T)
    wfcbd = np.zeros((128, 128), dtype=np.float32)
    wfcbd[:64, :64] = W_fc.T
    wfcbd[64:, 64:] = W_fc.T
    wfcbd = wfcbd.astype(ml_dtypes.bfloat16)

    # per-core qn_aug [65, IQ]: rows h*16+e = qn over the i-shard, row 64 = 1
    in_maps = []
    for core in range(N_CORES):
        ish = slice(core * I_SH, (core + 1) * I_SH)
        qa = np.zeros((65, IQ), dtype=np.float32)
        # qn[ish] : [16, h, q, e] -> [h*16+e, i*32+q]
        qa[:64, :] = qn[ish].transpose(1, 3, 0, 2).reshape(64, IQ)
        qa[64, :] = 1.0
        in_maps.append(
            {
                "qn_aug": qa,
                "kb": kb_pm,
                "vb": vb_pm,
                "onesbd": onesbd,
                "wfcbd": wfcbd,
            }
        )
    return in_maps


_NC_CACHE = {}


def _get_nc():
    if "nc" not in _NC_CACHE:
        _NC_CACHE["nc"] = build_kernel_nc()
    return _NC_CACHE["nc"]


def kernel(query, key, key_padding_mask, W_Q, W_K, W_V, W_fc):
    in_maps = _prep_inputs(query, key, key_padding_mask, W_Q, W_K, W_V, W_fc)
    nc = _get_nc()
    res = run_bass_kernel_spmd(nc, in_maps, list(range(N_CORES)))
    outs = []
    for c in range(N_CORES):
        raw = np.asarray(res.results[c]["out"])  # [T/2, 128, 512]
        # axes: (tp, (par, e), (i, q)) -> out[i, t = 2*tp + par, q, e]
        raw = raw.reshape(T // 2, 2, D_MODEL, I_SH, Q)
        outs.append(raw.transpose(3, 0, 1, 4, 2).reshape(I_SH, T, Q, D_MODEL))
    return np.concatenate(outs, axis=0)


if __name__ == "__main__":
    # smoke test against a numpy reference
    rng = np.random.default_rng(0)
    inputs = {
        "query": rng.standard_normal((I, Q, D_MODEL), dtype=np.float32),
        "key": rng.standard_normal((T, K, D_MODEL), dtype=np.float32),
        "key_padding_mask": rng.integers(0, 2, size=(T, K)).astype(bool),
        "W_Q": rng.standard_normal((HD, HD), dtype=np.float32) * 0.125,
        "W_K": rng.standard_normal((HD, HD), dtype=np.float32) * 0.125,
        "W_V": rng.standard_normal((HD, HD), dtype=np.float32) * 0.125,
        "W_fc": rng.standard_normal((D_MODEL, D_MODEL), dtype=np.float32) * 0.125,
    }
    out = kernel(**inputs)
    print("out", out.shape, out.dtype)
